# revision 1
# baseline (speedup 1.0000x reference)
"""Trainium2 Bass kernel for nn_CaptionDecoder (embedding -> masked LSTM -> vocab projection).

Sharding: the LSTM (B=32, S=64, H=512) is replicated on all 8 cores; the
vocab dimension of W_out/b_out is sharded 8-way (4000 per core). Each core
emits logits [S*B, 4000]; the host concatenates along vocab.

Device dataflow per core:
  - gather token embeddings via indirect DMA (128 tokens/tile, t-major order)
  - PE-transpose them to emb_T [e, tok] layout
  - per 4-step group: xg = W_x.T-free matmul into a PSUM block [128=4*32, 2048],
    + bias via a K=1 rank-1 matmul
  - each LSTM step s accumulates h_{t-1} @ W_h INTO rows [32s:32s+32] of that
    same PSUM block (base-partition offset matmul), so z = xg + h@W_h + b is
    materialized with zero extra copies
  - gates: ScalarE sigmoid/tanh straight out of PSUM; state update + Keras
    zero-token masking on VectorE (copy_predicated with a [32,1] mask column)
  - h is PE-transposed back each step into a [128, 8*32] ring that serves as
    the stationary lhsT for both the next steps and the group's logits matmul
  - logits: W_out streamed as the moving operand against the 4-step h block
    (M=128), bias via K=1 matmul, ScalarE copy to SBUF, DMA out.
"""

import sys

import numpy as np

if "/opt/trn_rl_repo" not in sys.path:
    sys.path.insert(0, "/opt/trn_rl_repo")

import concourse.bass as bass
import concourse.bacc as bacc
import concourse.mybir as mybir
import concourse.tile as tile
from concourse.bass_utils import run_bass_kernel_spmd
from concourse.masks import make_identity

VOCAB, EMBED, HIDDEN, CTX = 32000, 512, 512, 2048
B, S = 32, 64
G4 = 4 * HIDDEN  # 2048 gate width
NCORES = 8
VSH = VOCAB // NCORES  # 4000 vocab per core
P = 128
T = S * B  # 2048 tokens, t-major (tok = t*B + b)
NT = T // P  # 16 token tiles / groups
NK = HIDDEN // P  # 4 k-chunks over hidden/embed
NKC = CTX // P  # 16 k-chunks over context
NV = 8  # vocab slices per core
VS = VSH // NV  # 500 wide each
F32 = mybir.dt.float32
BF = mybir.dt.bfloat16
I32 = mybir.dt.int32

_CACHE: dict = {}



def _build_program() -> bass.Bass:
    nc = bacc.Bacc(None)

    ctx_d = nc.declare_dram_parameter("context_t", [CTX, B], BF, isOutput=False)
    embt_d = nc.declare_dram_parameter("emb_t", [EMBED, T], BF, isOutput=False)
    wih_d = nc.declare_dram_parameter("w_ih", [CTX, HIDDEN], BF, isOutput=False)
    wic_d = nc.declare_dram_parameter("w_ic", [CTX, HIDDEN], BF, isOutput=False)
    wx_d = nc.declare_dram_parameter("w_x", [EMBED, G4], BF, isOutput=False)
    wh_d = nc.declare_dram_parameter("w_h", [HIDDEN, G4], BF, isOutput=False)
    bg_d = nc.declare_dram_parameter("b_g", [G4], BF, isOutput=False)
    bih_d = nc.declare_dram_parameter("b_ih", [HIDDEN], BF, isOutput=False)
    bic_d = nc.declare_dram_parameter("b_ic", [HIDDEN], BF, isOutput=False)
    wout_d = nc.declare_dram_parameter("w_out", [HIDDEN, VSH], BF, isOutput=False)
    bout_d = nc.declare_dram_parameter("b_out", [VSH], BF, isOutput=False)
    mask_d = nc.declare_dram_parameter("maskf", [B, S], mybir.dt.uint8, isOutput=False)
    out_d = nc.declare_dram_parameter("logits", [T, VSH], F32, isOutput=True)

    with tile.TileContext(nc) as tc:
        with (
            tc.tile_pool(name="const", bufs=1) as cp,
            tc.tile_pool(name="stream", bufs=2) as sp,
            tc.tile_pool(name="embp", bufs=2) as ep,
            tc.tile_pool(name="state", bufs=1) as st,
            tc.tile_pool(name="gates", bufs=1) as gp,
            tc.tile_pool(name="lout", bufs=2) as lp,
            tc.tile_pool(name="pz", bufs=1, space="PSUM") as pz,
            tc.tile_pool(name="pa", bufs=2, space="PSUM") as pa,
            tc.tile_pool(name="pb", bufs=2, space="PSUM") as pb,
        ):
            # ---- resident constants / weights ----
            ident = cp.tile([P, P], F32, tag="ident", name="ident")
            make_identity(nc, ident[:])
            ones1 = cp.tile([1, P], BF, tag="ones1", name="ones1")
            nc.vector.memset(ones1[:], 1.0)

            ctx_sb = cp.tile([P, NKC * B], BF, tag="ctx", name="ctx")
            nc.sync.dma_start(
                out=ctx_sb[:].rearrange("p (k b) -> p k b", b=B),
                in_=ctx_d.rearrange("(k p) b -> p k b", p=P),
            )
            mask_sb = cp.tile([B, S], mybir.dt.uint8, tag="mask", name="mask")
            nc.sync.dma_start(out=mask_sb[:], in_=mask_d[:, :])
            bg_sb = cp.tile([1, G4], BF, tag="bg", name="bg")
            nc.sync.dma_start(out=bg_sb[:], in_=bg_d[None, :])
            bout_sb = cp.tile([1, VSH], BF, tag="bout", name="bout")
            nc.sync.dma_start(out=bout_sb[:], in_=bout_d[None, :])
            bih_sb = cp.tile([1, HIDDEN], BF, tag="bih", name="bih")
            nc.sync.dma_start(out=bih_sb[:], in_=bih_d[None, :])
            bic_sb = cp.tile([1, HIDDEN], BF, tag="bic", name="bic")
            nc.sync.dma_start(out=bic_sb[:], in_=bic_d[None, :])

            wh_sb = []
            wx_sb = []
            wout_sb = []
            for k in range(NK):
                t_wh = cp.tile([P, G4], BF, tag=f"wh{k}", name=f"wh{k}")
                nc.sync.dma_start(out=t_wh[:], in_=wh_d[k * P : (k + 1) * P, :])
                wh_sb.append(t_wh)
                t_wx = cp.tile([P, G4], BF, tag=f"wx{k}", name=f"wx{k}")
                nc.sync.dma_start(out=t_wx[:], in_=wx_d[k * P : (k + 1) * P, :])
                wx_sb.append(t_wx)
                t_wo = cp.tile([P, VSH], BF, tag=f"wout{k}", name=f"wout{k}")
                nc.sync.dma_start(out=t_wo[:], in_=wout_d[k * P : (k + 1) * P, :])
                wout_sb.append(t_wo)

            # ---- initial state h0/c0 = tanh(context @ W) ----
            # out [b=32, h=512]: lhsT = context_T chunk [128, 32] (stationary),
            # rhs = W_ih chunk [128, 512] streamed from DRAM.
            h_st = [st.tile([B, HIDDEN], F32, tag=f"h{i}", name=f"h{i}") for i in range(2)]
            c_st = [st.tile([B, HIDDEN], F32, tag=f"c{i}", name=f"c{i}") for i in range(2)]
            for w_dram, b_sb, dst in (
                (wih_d, bih_sb, h_st[0]),
                (wic_d, bic_sb, c_st[0]),
            ):
                ps = pb.tile([B, HIDDEN], F32, tag="pbt", name="pbt")
                for kc in range(NKC):
                    wt = sp.tile([P, HIDDEN], BF, tag="wstream", name="wstream")
                    nc.sync.dma_start(out=wt[:], in_=w_dram[kc * P : (kc + 1) * P, :])
                    nc.tensor.matmul(
                        out=ps[:],
                        lhsT=(ctx_sb[:, kc * B : (kc + 1) * B]),
                        rhs=(wt[:]),
                        start=(kc == 0),
                        stop=False,
                    )
                nc.tensor.matmul(
                    out=ps[:],
                    lhsT=(ones1[:1, :B]),
                    rhs=(b_sb[:1, :]),
                    start=False,
                    stop=True,
                )
                nc.scalar.activation(dst[:], ps[:], mybir.ActivationFunctionType.Tanh)

            # h transpose ring: slot(t) = t % 8 holds h_t as [h, b] column block;
            # groups alternate halves so each group's 4 slots form a [128,128] lhsT.
            ring = [cp.tile([P, 8 * B], BF, tag=f"ring{k}", name=f"ring{k}") for k in range(NK)]
            h0T = [cp.tile([P, B], BF, tag=f"h0T{k}", name=f"h0T{k}") for k in range(NK)]

            def transpose_h(src, dests):
                # src [32, 512] -> dests[k][:, col_slice] = src[:, k*128:+128].T
                for k in range(NK):
                    tp = pb.tile([P, B], F32, tag="pbt", name="pbt")
                    nc.tensor.transpose(
                        out=tp[:],
                        in_=src[:, k * P : (k + 1) * P],
                        identity=ident[:B, :B],
                    )
                    nc.vector.tensor_copy(dests[k], tp[:])

            transpose_h(h_st[0][:], [h0T[k][:, :] for k in range(NK)])

            # ---- pre-gathered, pre-transposed embeddings streamed per group ----
            def load_embT(g):
                ts = []
                for k in range(NK):
                    et = ep.tile([P, P], BF, tag=f"embT{k}", name=f"embT{k}")
                    nc.sync.dma_start(
                        out=et[:],
                        in_=embt_d[k * P : (k + 1) * P, g * P : (g + 1) * P],
                    )
                    ts.append(et)
                return ts

            embT_cur = load_embT(0)

            sig = mybir.ActivationFunctionType.Sigmoid
            tanh = mybir.ActivationFunctionType.Tanh

            for g in range(NT):
                # prefetch next group's embeddings
                embT_nxt = None
                if g + 1 < NT:
                    embT_nxt = load_embT(g + 1)

                # xg for this group into the shared PSUM block [128, 2048]
                xz = pz.tile([P, G4], F32, tag="xz", name="xz")
                for n in range(4):
                    ns = slice(n * HIDDEN, (n + 1) * HIDDEN)
                    for k in range(NK):
                        nc.tensor.matmul(
                            out=xz[:, ns],
                            lhsT=(embT_cur[k][:]),
                            rhs=(wx_sb[k][:, ns]),
                            start=(k == 0),
                            stop=False,
                        )
                    nc.tensor.matmul(
                        out=xz[:, ns],
                        lhsT=(ones1[:1, :]),
                        rhs=(bg_sb[:1, ns]),
                        start=False,
                        stop=True,
                    )

                # ---- 4 LSTM steps accumulating into rows of xz ----
                for s in range(4):
                    t = 4 * g + s
                    rows = slice(B * s, B * (s + 1))
                    if t == 0:
                        hT_prev = [h0T[k][:, :] for k in range(NK)]
                    else:
                        sl = ((t - 1) % 8) * B
                        hT_prev = [ring[k][:, sl : sl + B] for k in range(NK)]

                    for n in range(4):
                        ns = slice(n * HIDDEN, (n + 1) * HIDDEN)
                        for k in range(NK):
                            nc.tensor.matmul(
                                out=xz[rows, ns],
                                lhsT=(hT_prev[k]),
                                rhs=(wh_sb[k][:, ns]),
                                start=False,
                                stop=False,
                                tile_position=(0, B * s),
                                skip_group_check=True,
                            )

                    # gates from PSUM rows (Keras order i, f, g, o)
                    sig_i = gp.tile([B, HIDDEN], F32, tag="sig_i", name="sig_i")
                    sig_f = gp.tile([B, HIDDEN], F32, tag="sig_f", name="sig_f")
                    tanh_g = gp.tile([B, HIDDEN], F32, tag="tanh_g", name="tanh_g")
                    sig_o = gp.tile([B, HIDDEN], F32, tag="sig_o", name="sig_o")
                    nc.scalar.activation(sig_i[:], xz[rows, 0:HIDDEN], sig)
                    nc.scalar.activation(sig_f[:], xz[rows, HIDDEN : 2 * HIDDEN], sig)
                    nc.scalar.activation(
                        tanh_g[:], xz[rows, 2 * HIDDEN : 3 * HIDDEN], tanh
                    )
                    nc.scalar.activation(sig_o[:], xz[rows, 3 * HIDDEN : 4 * HIDDEN], sig)

                    h_prev = h_st[t % 2]
                    c_prev = c_st[t % 2]
                    h_next = h_st[(t + 1) % 2]
                    c_next = c_st[(t + 1) % 2]

                    c_new = gp.tile([B, HIDDEN], F32, tag="c_new", name="c_new")
                    tmp = gp.tile([B, HIDDEN], F32, tag="tmp", name="tmp")
                    nc.vector.tensor_mul(c_new[:], sig_f[:], c_prev[:])
                    nc.vector.tensor_mul(tmp[:], sig_i[:], tanh_g[:])
                    nc.vector.tensor_add(c_new[:], c_new[:], tmp[:])

                    tanh_c = gp.tile([B, HIDDEN], F32, tag="tanh_c", name="tanh_c")
                    nc.scalar.activation(tanh_c[:], c_new[:], tanh)
                    h_new = gp.tile([B, HIDDEN], F32, tag="h_new", name="h_new")
                    nc.vector.tensor_mul(h_new[:], sig_o[:], tanh_c[:])

                    # Keras masking: masked (token==0) steps carry prev state
                    m_bc = mask_sb[:, t : t + 1].to_broadcast([B, HIDDEN])
                    nc.vector.tensor_copy(c_next[:], c_prev[:])
                    nc.vector.copy_predicated(c_next[:], m_bc, c_new[:])
                    nc.vector.tensor_copy(h_next[:], h_prev[:])
                    nc.vector.copy_predicated(h_next[:], m_bc, h_new[:])

                    sl = (t % 8) * B
                    transpose_h(
                        h_next[:], [ring[k][:, sl : sl + B] for k in range(NK)]
                    )

                # ---- logits for this group: [128 tokens, VSH] ----
                half = (g % 2) * (4 * B)
                for v in range(NV):
                    vs = slice(v * VS, (v + 1) * VS)
                    pl = pa.tile([P, VS], F32, tag="pa", name="pa")
                    for k in range(NK):
                        nc.tensor.matmul(
                            out=pl[:],
                            lhsT=(ring[k][:, half : half + 4 * B]),
                            rhs=(wout_sb[k][:, vs]),
                            start=(k == 0),
                            stop=False,
                        )
                    nc.tensor.matmul(
                        out=pl[:],
                        lhsT=(ones1[:1, :]),
                        rhs=(bout_sb[:1, vs]),
                        start=False,
                        stop=True,
                    )
                    lo = lp.tile([P, VS], F32, tag="lo", name="lo")
                    nc.scalar.copy(lo[:], pl[:])
                    nc.sync.dma_start(
                        out=out_d[g * P : (g + 1) * P, vs], in_=lo[:]
                    )

                embT_cur = embT_nxt

    return nc


def _get_program() -> bass.Bass:
    if "nc" not in _CACHE:
        _CACHE["nc"] = _build_program()
    return _CACHE["nc"]


def prep_in_maps(inputs) -> list:
    import ml_dtypes

    bf16 = ml_dtypes.bfloat16
    tok = np.asarray(inputs["target_tokens"])
    ctx = np.asarray(inputs["context"], dtype=np.float32)
    emb_table = np.asarray(inputs["emb_table"], np.float32)
    w_out = np.asarray(inputs["W_out"], np.float32)
    b_out = np.asarray(inputs["b_out"], np.float32)

    mask = (tok != 0).astype(np.uint8)  # [B, S]
    tok_t = tok.T.reshape(-1).astype(np.int64)  # t*B + b token order
    emb_t = np.ascontiguousarray(emb_table[tok_t].T.astype(bf16))  # [EMBED, T]
    ctx_t = np.ascontiguousarray(ctx.T.astype(bf16))  # [CTX, B]

    shared = {
        "context_t": ctx_t,
        "emb_t": emb_t,
        "w_ih": np.ascontiguousarray(np.asarray(inputs["W_ih"]).astype(bf16)),
        "w_ic": np.ascontiguousarray(np.asarray(inputs["W_ic"]).astype(bf16)),
        "w_x": np.ascontiguousarray(np.asarray(inputs["W_x"]).astype(bf16)),
        "w_h": np.ascontiguousarray(np.asarray(inputs["W_h"]).astype(bf16)),
        "b_g": np.ascontiguousarray(np.asarray(inputs["b"]).astype(bf16)),
        "b_ih": np.ascontiguousarray(np.asarray(inputs["b_ih"]).astype(bf16)),
        "b_ic": np.ascontiguousarray(np.asarray(inputs["b_ic"]).astype(bf16)),
        "maskf": np.ascontiguousarray(mask),
    }
    in_maps = []
    for j in range(NCORES):
        m = dict(shared)
        m["w_out"] = np.ascontiguousarray(w_out[:, j * VSH : (j + 1) * VSH].astype(bf16))
        m["b_out"] = np.ascontiguousarray(b_out[j * VSH : (j + 1) * VSH].astype(bf16))
        in_maps.append(m)
    return in_maps


def kernel(**inputs: np.ndarray) -> np.ndarray:
    in_maps = prep_in_maps(inputs)
    nc = _get_program()
    if not nc.is_finalized():
        nc.finalize()

    import os

    trace = bool(os.environ.get("CAPDEC_TRACE"))
    kw = {}
    if trace:
        kw["trace"] = True
        tdir = os.environ.get("CAPDEC_TRACE_DIR")
        if tdir:
            os.makedirs(tdir, exist_ok=True)
            kw["tmpdir"] = tdir
    bkr = run_bass_kernel_spmd(nc, in_maps, list(range(NCORES)), **kw)
    _CACHE["last_results"] = bkr
    res = bkr.results
    parts = [res[j]["logits"].reshape(S, B, VSH) for j in range(NCORES)]
    full = np.concatenate(parts, axis=-1)  # [S, B, VOCAB]
    return np.ascontiguousarray(full.transpose(1, 0, 2))



# revision 6
# speedup vs baseline: 1.5304x; 1.5304x over previous
"""Trainium2 Bass kernel for nn_CaptionDecoder (embedding -> masked LSTM -> vocab projection).

Sharding: LSTM (B=32, S=64, H=512) replicated on all 8 cores; vocab dim of
W_out/b_out sharded 8-way (4000 per core). Each core emits logits
[T=2048, 4000] bf16; host concatenates along vocab and upcasts to f32.

v2 design notes (vs v1 baseline at ~1.23ms):
  - PE was the bottleneck and ran HAM-cold (median MM 414ns ~= K=4/8): every
    LSTM step's gate/state chain left a >3.4us PE idle gap. v2 interleaves
    independent work (previous group's logits matmuls, next group's xg
    matmuls) into each step's dependency gap to keep the PE warm and busy.
  - k-outer loop order everywhere so stationaries are reused (4x fewer
    LDWEIGHTS).
  - Gate order in the 4H dim is host-permuted to [g | i | f | o] so one
    sigmoid covers i,f,o contiguously and tanh(g) is issued first.
  - Keras zero-token masking: c-carry is folded into the gates via a
    host-computed rank-1 matmul adding (1-m)*(-30) to z_i and (1-m)*(+30)
    to z_f (sigmoid saturates -> c_new = c_prev exactly within fp tol).
    h-carry merge ops are emitted only for steps that actually contain a
    masked token (program is built after inspecting the input tokens).
  - All embeddings resident in SBUF (one up-front DMA), logits written as
    one [128, 4000] bf16 DMA per token group.
"""

import sys

import numpy as np

if "/opt/trn_rl_repo" not in sys.path:
    sys.path.insert(0, "/opt/trn_rl_repo")

import concourse.bass as bass
import concourse.bacc as bacc
import concourse.mybir as mybir
import concourse.tile as tile
from concourse.bass_utils import run_bass_kernel_spmd
from concourse.masks import make_identity

VOCAB, EMBED, HIDDEN, CTX = 32000, 512, 512, 2048
B, S = 32, 64
G4 = 4 * HIDDEN  # 2048 gate width
NCORES = 8
VSH = VOCAB // NCORES  # 4000 vocab per core
P = 128
T = S * B  # 2048 tokens, t-major (tok = t*B + b)
NT = T // P  # 16 token tiles / groups
NK = HIDDEN // P  # 4 k-chunks over hidden/embed
NKC = CTX // P  # 16 k-chunks over context
NV = 8  # vocab slices per core
VS = VSH // NV  # 500 wide each
NPAIR = 4  # logits processed in pairs of v-slices
F32 = mybir.dt.float32
BF = mybir.dt.bfloat16

# gate order after host permutation: [g | i | f | o]
GS_G = slice(0, HIDDEN)
GS_I = slice(HIDDEN, 2 * HIDDEN)
GS_F = slice(2 * HIDDEN, 3 * HIDDEN)
GS_O = slice(3 * HIDDEN, 4 * HIDDEN)
GS_IFO = slice(HIDDEN, 4 * HIDDEN)

_CACHE: dict = {}


def _build_program(masked_steps: tuple, masked_groups: tuple) -> bass.Bass:
    nc = bacc.Bacc(None)

    ctx_d = nc.declare_dram_parameter("context_t", [CTX, B], BF, isOutput=False)
    embt_d = nc.declare_dram_parameter("emb_t", [P, NK * T], BF, isOutput=False)
    wih_d = nc.declare_dram_parameter("w_ih", [CTX, HIDDEN], BF, isOutput=False)
    wic_d = nc.declare_dram_parameter("w_ic", [CTX, HIDDEN], BF, isOutput=False)
    wx_d = nc.declare_dram_parameter("w_x", [EMBED, G4], BF, isOutput=False)
    wh_d = nc.declare_dram_parameter("w_h", [HIDDEN, G4], BF, isOutput=False)
    bg_d = nc.declare_dram_parameter("b_g", [G4], BF, isOutput=False)
    bih_d = nc.declare_dram_parameter("b_ih", [HIDDEN], BF, isOutput=False)
    bic_d = nc.declare_dram_parameter("b_ic", [HIDDEN], BF, isOutput=False)
    wout_d = nc.declare_dram_parameter("w_out", [HIDDEN, VSH], BF, isOutput=False)
    bout_d = nc.declare_dram_parameter("b_out", [VSH], BF, isOutput=False)
    mrow_d = nc.declare_dram_parameter("mrow", [NT, P], BF, isOutput=False)
    mask_d = nc.declare_dram_parameter("maskf", [B, S], mybir.dt.uint8, isOutput=False)
    out_d = nc.declare_dram_parameter("logits", [T, VSH], BF, isOutput=True)

    sig = mybir.ActivationFunctionType.Sigmoid
    tanh = mybir.ActivationFunctionType.Tanh

    with tile.TileContext(nc) as tc:
        with (
            tc.tile_pool(name="const", bufs=1) as cp,
            tc.tile_pool(name="stream", bufs=2) as sp,
            tc.tile_pool(name="state", bufs=1) as st,
            tc.tile_pool(name="gates", bufs=2) as gp,
            tc.tile_pool(name="lout", bufs=2) as lp,
            tc.tile_pool(name="pz", bufs=1, space="PSUM") as pz,
            tc.tile_pool(name="pa", bufs=2, space="PSUM") as pa,
            tc.tile_pool(name="pb", bufs=2, space="PSUM") as pb,
        ):
            # ---- startup-critical constants ----
            ident = cp.tile([P, P], BF, tag="ident", name="ident")
            make_identity(nc, ident[:])
            ones1 = cp.tile([1, P], BF, tag="ones1", name="ones1")
            nc.vector.memset(ones1[:], 1.0)

            ctx_sb = cp.tile([P, NKC * B], BF, tag="ctx", name="ctx")
            nc.sync.dma_start(
                out=ctx_sb[:].rearrange("p (k b) -> p k b", b=B),
                in_=ctx_d.rearrange("(k p) b -> p k b", p=P),
            )
            bih_sb = cp.tile([1, HIDDEN], BF, tag="bih", name="bih")
            nc.sync.dma_start(out=bih_sb[:], in_=bih_d[None, :])
            bic_sb = cp.tile([1, HIDDEN], BF, tag="bic", name="bic")
            nc.sync.dma_start(out=bic_sb[:], in_=bic_d[None, :])

            # ---- initial state h0/c0 = tanh(context @ W), streamed weights ----
            h_st = [st.tile([B, HIDDEN], BF, tag=f"h{i}", name=f"h{i}") for i in range(2)]
            c_st = [st.tile([B, HIDDEN], BF, tag=f"c{i}", name=f"c{i}") for i in range(2)]
            for w_dram, b_sb, dst in (
                (wih_d, bih_sb, h_st[0]),
                (wic_d, bic_sb, c_st[0]),
            ):
                ps = pb.tile([B, HIDDEN], F32, tag="pbt", name="pbt")
                for kc in range(NKC):
                    wt = sp.tile([P, HIDDEN], BF, tag="wstream", name="wstream")
                    nc.sync.dma_start(out=wt[:], in_=w_dram[kc * P : (kc + 1) * P, :])
                    nc.tensor.matmul(
                        out=ps[:],
                        lhsT=(ctx_sb[:, kc * B : (kc + 1) * B]),
                        rhs=(wt[:]),
                        start=(kc == 0),
                        stop=False,
                    )
                nc.tensor.matmul(
                    out=ps[:],
                    lhsT=(ones1[:1, :B]),
                    rhs=(b_sb[:1, :]),
                    start=False,
                    stop=True,
                )
                nc.scalar.activation(dst[:], ps[:], tanh)

            # ---- resident weights, ordered by first use ----
            bg_sb = cp.tile([1, G4], BF, tag="bg", name="bg")
            nc.sync.dma_start(out=bg_sb[:], in_=bg_d[None, :])
            wx_sb = []
            for k in range(NK):
                t_wx = cp.tile([P, G4], BF, tag=f"wx{k}", name=f"wx{k}")
                nc.sync.dma_start(out=t_wx[:], in_=wx_d[k * P : (k + 1) * P, :])
                wx_sb.append(t_wx)
            # all pre-gathered transposed embeddings resident: [128, k, tok]
            embT = cp.tile([P, NK * T], BF, tag="embT", name="embT")
            nc.sync.dma_start(out=embT[:], in_=embt_d[:, :])
            mrow_sb = cp.tile([NT, P], BF, tag="mrow", name="mrow")
            nc.sync.dma_start(out=mrow_sb[:], in_=mrow_d[:, :])
            mbias = cp.tile([1, G4], BF, tag="mbias", name="mbias")
            nc.vector.memset(mbias[:], 0.0)
            nc.vector.memset(mbias[:, GS_I], -30.0)
            nc.vector.memset(mbias[:, GS_F], 30.0)
            wh_sb = []
            for k in range(NK):
                t_wh = cp.tile([P, G4], BF, tag=f"wh{k}", name=f"wh{k}")
                nc.sync.dma_start(out=t_wh[:], in_=wh_d[k * P : (k + 1) * P, :])
                wh_sb.append(t_wh)
            mask_sb = cp.tile([B, S], mybir.dt.uint8, tag="mask", name="mask")
            nc.sync.dma_start(out=mask_sb[:], in_=mask_d[:, :])
            bout_sb = cp.tile([1, VSH], BF, tag="bout", name="bout")
            nc.sync.dma_start(out=bout_sb[:], in_=bout_d[None, :])
            wout_sb = []
            for k in range(NK):
                t_wo = cp.tile([P, VSH], BF, tag=f"wout{k}", name=f"wout{k}")
                nc.sync.dma_start(out=t_wo[:], in_=wout_d[k * P : (k + 1) * P, :])
                wout_sb.append(t_wo)

            # h transpose ring: slot(t) = t % 8 holds h_t as [h, b] column block;
            # groups alternate halves so each group's 4 slots form a [128,128] lhsT.
            ring = [cp.tile([P, 8 * B], BF, tag=f"ring{k}", name=f"ring{k}") for k in range(NK)]
            h0T = [cp.tile([P, B], BF, tag=f"h0T{k}", name=f"h0T{k}") for k in range(NK)]

            def transpose_h(src, dests):
                # src [32, 512] -> dests[k] = src[:, k*128:+128].T  ([128, 32])
                for k in range(NK):
                    tp = pb.tile([P, B], BF, tag="pbt", name="pbt")
                    nc.tensor.transpose(
                        out=tp[:],
                        in_=src[:, k * P : (k + 1) * P],
                        identity=ident[:B, :B],
                    )
                    if k % 2 == 0:
                        nc.vector.tensor_copy(dests[k], tp[:])
                    else:
                        nc.scalar.copy(dests[k], tp[:])

            transpose_h(h_st[0][:], [h0T[k][:, :] for k in range(NK)])

            def xg_mms(g, xz):
                # xg for group g into PSUM block [128, 2048], k-outer
                for k in range(NK):
                    lhs = embT[:, k * T + g * P : k * T + (g + 1) * P]
                    for n in range(4):
                        ns = slice(n * HIDDEN, (n + 1) * HIDDEN)
                        nc.tensor.matmul(
                            out=xz[:, ns],
                            lhsT=lhs,
                            rhs=(wx_sb[k][:, ns]),
                            start=(k == 0),
                            stop=False,
                        )
                if g in masked_groups:
                    # rank-1: z_i += (1-m)*(-30), z_f += (1-m)*(+30)
                    for ns in (GS_I, GS_F):
                        nc.tensor.matmul(
                            out=xz[:, ns],
                            lhsT=(mrow_sb[g : g + 1, :]),
                            rhs=(mbias[:1, ns]),
                            start=False,
                            stop=False,
                        )
                for n in range(4):
                    ns = slice(n * HIDDEN, (n + 1) * HIDDEN)
                    nc.tensor.matmul(
                        out=xz[:, ns],
                        lhsT=(ones1[:1, :]),
                        rhs=(bg_sb[:1, ns]),
                        start=False,
                        stop=True,
                    )

            def logits_pair(gprev, p, lo):
                # v-slices (2p, 2p+1) of group gprev into lo[:, 1000p:1000p+1000]
                half = (gprev % 2) * (4 * B)
                pls = [
                    pa.tile([P, VS], F32, tag="pa", name="pa"),
                    pa.tile([P, VS], F32, tag="pa", name="pa"),
                ]
                for k in range(NK):
                    lhs = ring[k][:, half : half + 4 * B]
                    for j in range(2):
                        vs = slice((2 * p + j) * VS, (2 * p + j + 1) * VS)
                        nc.tensor.matmul(
                            out=pls[j][:],
                            lhsT=lhs,
                            rhs=(wout_sb[k][:, vs]),
                            start=(k == 0),
                            stop=False,
                        )
                for j in range(2):
                    vs = slice((2 * p + j) * VS, (2 * p + j + 1) * VS)
                    nc.tensor.matmul(
                        out=pls[j][:],
                        lhsT=(ones1[:1, :]),
                        rhs=(bout_sb[:1, vs]),
                        start=False,
                        stop=True,
                    )
                for j in range(2):
                    dst = lo[:, (2 * p + j) * VS : (2 * p + j + 1) * VS]
                    if j == 0:
                        nc.scalar.copy(dst, pls[j][:])
                    else:
                        nc.vector.tensor_copy(dst, pls[j][:])

            # ---- main loop over 16 token groups of 4 steps ----
            xz = pz.tile([P, G4], F32, tag="xz", name="xz")
            xg_mms(0, xz)

            for g in range(NT):
                if g >= 1:
                    lo = lp.tile([P, VSH], BF, tag="lo", name="lo")
                for s in range(4):
                    t = 4 * g + s
                    rows = slice(B * s, B * (s + 1))
                    if t == 0:
                        hT_prev = [h0T[k][:, :] for k in range(NK)]
                    else:
                        sl = ((t - 1) % 8) * B
                        hT_prev = [ring[k][:, sl : sl + B] for k in range(NK)]

                    # recurrence matmuls for step t (k-outer: 4 LDW, 16 MM)
                    for k in range(NK):
                        for n in range(4):
                            ns = slice(n * HIDDEN, (n + 1) * HIDDEN)
                            nc.tensor.matmul(
                                out=xz[rows, ns],
                                lhsT=(hT_prev[k]),
                                rhs=(wh_sb[k][:, ns]),
                                start=False,
                                stop=False,
                                tile_position=(0, B * s),
                                skip_group_check=True,
                            )

                    # PE filler while ScalarE/VectorE run this step's chain:
                    # one logits pair of the previous group
                    if g >= 1:
                        logits_pair(g - 1, s, lo)

                    # ---- gates (Keras order i,f,g,o -> stored [g|i|f|o]) ----
                    tanh_g = gp.tile([B, HIDDEN], BF, tag="tanh_g", name="tanh_g")
                    nc.scalar.activation(tanh_g[:], xz[rows, GS_G], tanh)
                    sig_ifo = gp.tile([B, 3 * HIDDEN], BF, tag="sig_ifo", name="sig_ifo")
                    nc.scalar.activation(sig_ifo[:], xz[rows, GS_IFO], sig)
                    sig_i = sig_ifo[:, 0:HIDDEN]
                    sig_f = sig_ifo[:, HIDDEN : 2 * HIDDEN]
                    sig_o = sig_ifo[:, 2 * HIDDEN : 3 * HIDDEN]

                    # next group's xg: issued after this group's last gate
                    # reads of xz (pz bufs=1 reuse is ordered by program order)
                    if s == 3 and g + 1 < NT:
                        xz_n = pz.tile([P, G4], F32, tag="xz", name="xz")
                        xg_mms(g + 1, xz_n)

                    h_prev = h_st[t % 2]
                    c_prev = c_st[t % 2]
                    h_next = h_st[(t + 1) % 2]
                    c_next = c_st[(t + 1) % 2]
                    tanh_c = gp.tile([B, HIDDEN], BF, tag="tanh_c", name="tanh_c")
                    tmp = gp.tile([B, HIDDEN], BF, tag="tmp", name="tmp")

                    if t in masked_steps:
                        # rare: some batch rows masked at step t -> carry h,c
                        c_new = gp.tile([B, HIDDEN], BF, tag="c_new", name="c_new")
                        h_new = gp.tile([B, HIDDEN], BF, tag="h_new", name="h_new")
                        nc.vector.tensor_mul(c_new[:], sig_f, c_prev[:])
                        nc.vector.tensor_mul(tmp[:], sig_i, tanh_g[:])
                        nc.vector.tensor_add(c_new[:], c_new[:], tmp[:])
                        nc.scalar.activation(tanh_c[:], c_new[:], tanh)
                        nc.vector.tensor_mul(h_new[:], sig_o, tanh_c[:])
                        m_bc = mask_sb[:, t : t + 1].to_broadcast([B, HIDDEN])
                        nc.vector.tensor_copy(c_next[:], c_prev[:])
                        nc.vector.copy_predicated(c_next[:], m_bc, c_new[:])
                        nc.vector.tensor_copy(h_next[:], h_prev[:])
                        nc.vector.copy_predicated(h_next[:], m_bc, h_new[:])
                    else:
                        # c-carry for masked rows is already exact via the
                        # gate-bias matmul; write state in place
                        nc.vector.tensor_mul(c_next[:], sig_f, c_prev[:])
                        nc.vector.tensor_mul(tmp[:], sig_i, tanh_g[:])
                        nc.vector.tensor_add(c_next[:], c_next[:], tmp[:])
                        nc.scalar.activation(tanh_c[:], c_next[:], tanh)
                        nc.vector.tensor_mul(h_next[:], sig_o, tanh_c[:])

                    sl = (t % 8) * B
                    transpose_h(
                        h_next[:], [ring[k][:, sl : sl + B] for k in range(NK)]
                    )

                # this group's logits tile complete -> one DMA out
                if g >= 1:
                    nc.sync.dma_start(
                        out=out_d[(g - 1) * P : g * P, :], in_=lo[:]
                    )
                if g + 1 < NT:
                    xz = xz_n

            # tail: logits for the last group
            lo = lp.tile([P, VSH], BF, tag="lo", name="lo")
            for p in range(NPAIR):
                logits_pair(NT - 1, p, lo)
            nc.sync.dma_start(out=out_d[(NT - 1) * P : NT * P, :], in_=lo[:])

    return nc


def _get_program(masked_steps: tuple, masked_groups: tuple) -> bass.Bass:
    key = ("v2", masked_steps, masked_groups)
    if _CACHE.get("key") != key:
        nc = _build_program(masked_steps, masked_groups)
        nc.finalize()
        _CACHE["key"] = key
        _CACHE["nc"] = nc
    return _CACHE["nc"]


def prep_in_maps(inputs):
    import ml_dtypes

    bf16 = ml_dtypes.bfloat16
    tok = np.asarray(inputs["target_tokens"])
    ctx = np.asarray(inputs["context"], dtype=np.float32)
    emb_table = np.asarray(inputs["emb_table"], np.float32)
    w_out = np.asarray(inputs["W_out"], np.float32)
    b_out = np.asarray(inputs["b_out"], np.float32)

    mask = (tok != 0).astype(np.uint8)  # [B, S]
    tok_t = tok.T.reshape(-1)  # t*B + b token order
    emb_g = emb_table[tok_t].astype(bf16)  # [T, EMBED]
    # [128, k, tok] layout: emb_t[p, k*T + t] = emb_g[t, 128k+p]
    emb_t = np.ascontiguousarray(
        emb_g.T.reshape(NK, P, T).transpose(1, 0, 2).reshape(P, NK * T)
    )
    ctx_t = np.ascontiguousarray(ctx.T.astype(bf16))  # [CTX, B]

    # permute gate blocks [i|f|g|o] -> [g|i|f|o]
    perm = np.concatenate(
        [
            np.arange(2 * HIDDEN, 3 * HIDDEN),  # g
            np.arange(0, HIDDEN),  # i
            np.arange(HIDDEN, 2 * HIDDEN),  # f
            np.arange(3 * HIDDEN, 4 * HIDDEN),  # o
        ]
    )
    w_x = np.asarray(inputs["W_x"], np.float32)[:, perm]
    w_h = np.asarray(inputs["W_h"], np.float32)[:, perm]
    b_g = np.asarray(inputs["b"], np.float32)[perm]

    # (1 - mask) per token, t-major, grouped [NT, 128]
    mrow = (1.0 - mask.T.reshape(-1).astype(np.float32)).reshape(NT, P)

    shared = {
        "context_t": ctx_t,
        "emb_t": emb_t,
        "w_ih": np.ascontiguousarray(np.asarray(inputs["W_ih"]).astype(bf16)),
        "w_ic": np.ascontiguousarray(np.asarray(inputs["W_ic"]).astype(bf16)),
        "w_x": np.ascontiguousarray(w_x.astype(bf16)),
        "w_h": np.ascontiguousarray(w_h.astype(bf16)),
        "b_g": np.ascontiguousarray(b_g.astype(bf16)),
        "b_ih": np.ascontiguousarray(np.asarray(inputs["b_ih"]).astype(bf16)),
        "b_ic": np.ascontiguousarray(np.asarray(inputs["b_ic"]).astype(bf16)),
        "mrow": np.ascontiguousarray(mrow.astype(bf16)),
        "maskf": np.ascontiguousarray(mask),
    }
    in_maps = []
    for j in range(NCORES):
        m = dict(shared)
        m["w_out"] = np.ascontiguousarray(w_out[:, j * VSH : (j + 1) * VSH].astype(bf16))
        m["b_out"] = np.ascontiguousarray(b_out[j * VSH : (j + 1) * VSH].astype(bf16))
        in_maps.append(m)

    # program specialization on the mask pattern
    col_any = mask.min(axis=0) == 0  # step t has any masked row
    masked_steps = tuple(int(t) for t in np.nonzero(col_any)[0])
    masked_groups = tuple(sorted({t // 4 for t in masked_steps}))
    return in_maps, masked_steps, masked_groups


def kernel(**inputs: np.ndarray) -> np.ndarray:
    in_maps, masked_steps, masked_groups = prep_in_maps(inputs)
    nc = _get_program(masked_steps, masked_groups)

    import os

    trace = bool(os.environ.get("CAPDEC_TRACE"))
    kw = {}
    if trace:
        kw["trace"] = True
        tdir = os.environ.get("CAPDEC_TRACE_DIR")
        if tdir:
            os.makedirs(tdir, exist_ok=True)
            kw["tmpdir"] = tdir
    bkr = run_bass_kernel_spmd(nc, in_maps, list(range(NCORES)), **kw)
    _CACHE["last_results"] = bkr
    res = bkr.results
    parts = [res[j]["logits"].reshape(S, B, VSH) for j in range(NCORES)]
    full = np.concatenate(parts, axis=-1)  # [S, B, VOCAB] bf16
    return np.ascontiguousarray(full.transpose(1, 0, 2)).astype(np.float32)
